# revision 38
# baseline (speedup 1.0000x reference)
"""Difference 3D cost volume on 8 Trainium2 NeuronCores.

cost[n,c,d,h,w] = l[n,c,h,w] - r[n,c,h,w-d]  (w >= d), else 1.0
Shapes: l,r [2,32,128,256] f32 -> out [2,32,48,128,256] f32.

Sharding: data-parallel over the 64 (n,c) slices, 8 per core. Each core
computes, per slice, the [H, D, W] volume in CH-disparity chunks: one
fused tensor_sub per chunk (broadcast l over d via stride-0 AP, shift r
via stride -1 AP against a small zeroed left-pad) and one fully
contiguous store per chunk into a PACKED device output layout. The
chunk for disparities [d0, d0+CH) only computes and stores columns
w >= d0, trimming ~8% of HBM store traffic (the bottleneck) while
keeping every store a single contiguous run per partition; the host
gather unpacks chunks and writes the constant-1.0 w < d prefixes,
which cover every column the device skipped. r is loaded unpadded; an
8-column SBUF memset provides the shift headroom within a chunk.

CH=4 (96 stores of ~0.4-0.5 MB per pass) measurably beats CH=8 on HW
(~5-10%): finer DMAs multiplex the descriptor rings better and hide
each DMA's HBM completion-receipt tail; CH<=3 falls off a cliff.
Stores round-robin over three descriptor rings (SP HWDGE, ACT HWDGE,
and the SWDGE ring driven by the otherwise-idle GpSimd/POOL engine);
all subtracts run on DVE (~94 us busy, under the ~130 us DMA
roofline), and input loads alternate between the two HWDGE rings.
"""

import numpy as np

N, C, H, W, D = 2, 32, 128, 256, 48
NCORES = 8
PAIRS = N * C
PPC = PAIRS // NCORES  # (n,c) slices per core
CH = 4  # disparities per compute/store chunk (divides D)
RPAD = 8  # SBUF left-pad columns on r; must be > CH-1, 32B-aligned
OFFLOAD = 0  # if >0, every OFFLOAD-th chunk's subtract runs on GpSimd
SPLIT_STORES = True  # alternate stores between the SP and ACT HWDGE rings
STORE_RINGS = 3  # 2: SP+ACT HWDGE; 3: also the SWDGE (gpsimd) ring
LOAD_ENG = "alt"  # input loads alternate between the two HWDGE rings
WEDGE = True  # store only w >= d0+CH per chunk; host computes the wedge
LRBUFS = 6  # input-tile pool depth (slices prefetched ahead)

def _chunks(ch, wedge=False, align=False):
    """Packed chunk table [(d0, stored_width, packed_offset)], total cols.

    wedge=True narrows each chunk by ch columns (store only w >= d0+ch):
    the host computes the remaining per-row wedge (<= ch cols). Trimming
    by ch (not ch-1) keeps every store run a 64-byte multiple — trimming
    by ch-1 breaks alignment and measures ~15% SLOWER despite fewer
    bytes. align=True pads each chunk's slot to ch*W columns so every
    store's DRAM base is 4 KiB-aligned per partition row (same bytes
    written; the pad is a skipped hole).
    """
    table, off = [], 0
    for c in range(D // ch):
        d0 = c * ch
        wv = W - d0 - (ch if wedge else 0)
        table.append((d0, wv, off))
        off += ch * W if align else ch * wv
    return table, off


# Default-config table: PK = total packed cols (11040 for CH=4 wedge).
CHUNKS, PK = _chunks(CH, WEDGE)

_nc_cache = None
_runner_cache = None


def _emit(tc, lf, rf, out, ch=CH, offload=OFFLOAD, split_stores=SPLIT_STORES,
          load_eng=LOAD_ENG, obufs=None, store_rings=STORE_RINGS,
          wedge=WEDGE, split2=False, align=False, lrbufs=LRBUFS,
          ringrot=True):
    """Emit the per-core program. lf [PPC,H,W], rf [PPC,H,W],
    out [PPC,H,PK] packed."""
    from concourse import mybir
    from contextlib import ExitStack

    nc = tc.nc
    chunks, _pk = _chunks(ch, wedge, align)
    w0off = ch if wedge else 0  # first stored col is d0 + w0off
    with ExitStack() as ctx:
        lp = ctx.enter_context(tc.tile_pool(name="lp", bufs=lrbufs))
        rp = ctx.enter_context(tc.tile_pool(name="rp", bufs=lrbufs))
        op = ctx.enter_context(
            tc.tile_pool(name="op", bufs=obufs or (6 if ch <= 12 else 4))
        )
        g = 0  # global chunk counter (engine assignment round-robin)
        for p in range(PPC):
            le = (nc.scalar, nc.sync)[p % 2] if load_eng == "alt" else getattr(
                nc, load_eng
            )
            lt = lp.tile([H, W], mybir.dt.float32)
            le.dma_start(lt[:], lf[p])
            # wedge: r reads stay at col >= w0off-(ch-1) >= 1, so no
            # zeroed left-pad is needed and r rows stay 64B-aligned.
            rpad = 0 if wedge else RPAD
            rt = rp.tile([H, rpad + W], mybir.dt.float32)
            if rpad:
                nc.vector.memset(rt[:, 0:rpad], 0.0)
            le.dma_start(rt[:, rpad : rpad + W], rf[p])

            for c in range(D // ch):
                d0, wv, off = chunks[c]
                ot = op.tile([H, ch * wv], mybir.dt.float32)

                # ot[h, k*wv + j] = l[h, d0+w0off+j] - r[h, d0+w0off+j-(d0+k)]
                #                 = lt[h, d0+w0off+j] - rt[h, rpad+w0off+j-k]
                l_ap = lt[:, d0 + w0off : d0 + w0off + wv]
                l_ap.ap = l_ap.ap[:-1] + [[0, ch], [1, wv]]
                r_ap = rt[:, rpad + w0off : rpad + w0off + wv]
                r_ap.ap = r_ap.ap[:-1] + [[-1, ch], [1, wv]]
                o_ap = ot[:, 0 : ch * wv]
                o_ap.ap = o_ap.ap[:-1] + [[wv, ch], [1, wv]]
                eng = (
                    nc.gpsimd
                    if offload and g % offload == offload - 1
                    else nc.vector
                )
                g += 1
                eng.tensor_sub(o_ap, l_ap, r_ap)

                def ring(i):
                    if isinstance(store_rings, str):
                        key = store_rings[i % len(store_rings)]
                        return {"s": nc.scalar, "y": nc.sync, "g": nc.gpsimd}[key]
                    if store_rings == 3:
                        return (nc.scalar, nc.sync, nc.gpsimd)[i % 3]
                    return nc.scalar if split_stores and i % 2 else nc.sync

                # ringrot: rotate ring phase per slice so each ring gets an
                # equal byte total over the pass (chunk sizes shrink with c,
                # and 12 chunks/slice keeps g%3 in phase every slice).
                rg = g + p if ringrot else g
                if split2:
                    half = (ch * wv) // 2
                    ring(rg).dma_start(out[p][:, off : off + half], ot[:, 0:half])
                    ring(rg + 1).dma_start(
                        out[p][:, off + half : off + ch * wv],
                        ot[:, half : ch * wv],
                    )
                else:
                    ring(rg).dma_start(out[p][:, off : off + ch * wv], ot[:])


def _declare_io(nc, ch=CH):
    from concourse import mybir

    _table, pk = _chunks(ch, WEDGE)
    lf = nc.dram_tensor(
        "lf", [PPC, H, W], mybir.dt.float32, kind="ExternalInput"
    ).ap()
    rf = nc.dram_tensor(
        "rf", [PPC, H, W], mybir.dt.float32, kind="ExternalInput"
    ).ap()
    out = nc.dram_tensor(
        "out", [PPC, H, pk], mybir.dt.float32, kind="ExternalOutput"
    ).ap()
    return lf, rf, out


def _build():
    global _nc_cache
    if _nc_cache is not None:
        return _nc_cache
    import concourse.tile as tile
    from concourse import bacc

    nc = bacc.Bacc(
        "TRN2", target_bir_lowering=False, debug=False, num_devices=NCORES
    )
    lf, rf, out = _declare_io(nc)
    with tile.TileContext(nc) as tc:
        _emit(tc, lf, rf, out)
    nc.compile()
    _nc_cache = nc
    return nc


def _get_runner():
    """Build (once) a cached PJRT executable over the 8-core mesh.

    No donation: the zero output-operands stay resident on device and are
    reused every call; the host gather fills every byte the NEFF doesn't
    write, so stale result buffers are fine.
    """
    global _runner_cache
    if _runner_cache is not None:
        return _runner_cache

    import jax
    from jax.sharding import Mesh, NamedSharding, PartitionSpec

    import concourse.mybir as mybir
    from concourse.bass2jax import (
        _bass_exec_p,
        install_neuronx_cc_hook,
        partition_id_tensor,
    )

    try:
        from jax.experimental.shard_map import shard_map
    except ImportError:
        from jax.shard_map import shard_map

    nc = _build()
    install_neuronx_cc_hook()
    partition_name = nc.partition_id_tensor.name if nc.partition_id_tensor else None

    in_names, out_names, out_avals, zero_outs = [], [], [], []
    for alloc in nc.m.functions[0].allocations:
        if not isinstance(alloc, mybir.MemoryLocationSet):
            continue
        name = alloc.memorylocations[0].name
        if alloc.kind == "ExternalInput":
            if name != partition_name:
                in_names.append(name)
        elif alloc.kind == "ExternalOutput":
            shape = tuple(alloc.tensor_shape)
            dtype = mybir.dt.np(alloc.dtype)
            out_names.append(name)
            out_avals.append(jax.core.ShapedArray(shape, dtype))
            zero_outs.append(np.zeros(shape, dtype))
    all_in_names = list(in_names) + list(out_names)
    if partition_name is not None:
        all_in_names.append(partition_name)

    def _body(*args):
        operands = list(args)
        if partition_name is not None:
            operands.append(partition_id_tensor())
        outs = _bass_exec_p.bind(
            *operands,
            out_avals=tuple(out_avals),
            in_names=tuple(all_in_names),
            out_names=tuple(out_names),
            lowering_input_output_aliases=(),
            sim_require_finite=True,
            sim_require_nnan=True,
            nc=nc,
        )
        return tuple(outs)

    devices = jax.devices()[:NCORES]
    mesh = Mesh(np.asarray(devices), ("core",))
    nin = len(in_names)
    nout = len(out_names)
    fn = jax.jit(
        shard_map(
            _body,
            mesh=mesh,
            in_specs=(PartitionSpec("core"),) * (nin + nout),
            out_specs=(PartitionSpec("core"),) * nout,
            check_rep=False,
        ),
        keep_unused=True,
    )
    sharding = NamedSharding(mesh, PartitionSpec("core"))
    zeros_dev = [
        jax.device_put(
            np.zeros((NCORES * z.shape[0], *z.shape[1:]), z.dtype), sharding
        )
        for z in zero_outs
    ]
    _runner_cache = (fn, in_names, zeros_dev, sharding)
    return _runner_cache


def _prep_inputs(l_fmap, r_fmap):
    l = np.ascontiguousarray(np.asarray(l_fmap, dtype=np.float32)).reshape(
        PAIRS, H, W
    )
    r = np.ascontiguousarray(np.asarray(r_fmap, dtype=np.float32)).reshape(
        PAIRS, H, W
    )
    return {"lf": l, "rf": r}


def _gather(out_global, l, r):
    """Packed [PAIRS,H,PK] device result -> [N,C,D,H,W].

    Unpacks the device chunks (w >= d0+CH when WEDGE), computes the
    per-row wedge w in [d, d0+CH) from l/r on the host, and writes the
    constant-1.0 w < d prefixes."""
    packed = np.asarray(out_global).reshape(PAIRS, H, PK)
    out = np.empty((PAIRS, D, H, W), np.float32)
    w0off = CH if WEDGE else 0
    for d0, wv, off in CHUNKS:
        blk = packed[:, :, off : off + CH * wv].reshape(PAIRS, H, CH, wv)
        out[:, d0 : d0 + CH, :, d0 + w0off :] = blk.swapaxes(1, 2)
        if WEDGE:
            for k in range(CH):
                d = d0 + k
                out[:, d, :, d : d0 + CH] = (
                    l[:, :, d : d0 + CH] - r[:, :, 0 : d0 + CH - d]
                )
    for d in range(1, D):
        out[:, d, :, :d] = 1.0
    return out.reshape(N, C, D, H, W)


def kernel(l_fmap, r_fmap):
    import jax

    fn, in_names, zeros_dev, sharding = _get_runner()
    named = _prep_inputs(l_fmap, r_fmap)
    concat_in = [jax.device_put(named[name], sharding) for name in in_names]
    out_arrs = fn(*concat_in, *zeros_dev)
    return _gather(out_arrs[0], named["lf"], named["rf"])


def run(l_fmap, r_fmap, trace=False):
    """Legacy path via run_bass_kernel_spmd (used by test.py)."""
    from concourse.bass_utils import run_bass_kernel_spmd

    named = _prep_inputs(l_fmap, r_fmap)
    in_maps = [
        {k: np.ascontiguousarray(v[c * PPC : (c + 1) * PPC]) for k, v in named.items()}
        for c in range(NCORES)
    ]
    nc = _build()
    res = run_bass_kernel_spmd(
        nc, in_maps, core_ids=list(range(NCORES)), trace=trace
    )
    parts = [res.results[k]["out"] for k in range(NCORES)]
    out = _gather(np.concatenate(parts, axis=0), named["lf"], named["rf"])
    return out, res
